# revision 6
# baseline (speedup 1.0000x reference)
"""Trainium2 Bass kernel for the AdaptPrompt segment-reduce problem.

Computation (see reference):
    counts/centers/delta = per-class segment means over 10000 few-shot rows
    xr = Q1_x[remaining_idxes]                       # [190000, 256] gather
    sim = softmax(normalize(xr) @ normalize(centers).T)
    out = xr + sim @ delta

Strategy (row-major streaming, constant PE stationaries):
  Host dedups the remaining-row support (~61% of table rows are referenced)
  and value-range shards the unique rows across 8 cores; out[i] =
  dev_out[rem[i]] is applied on the host after the gather-free streaming
  kernel.

  Device pipeline per 256-row chunk:
    - logits  q[c, r] = cn8^T @ x8 on PE, fp8 DoubleRow, with the class
      matrix cn8T as the stationary operand (constant -> trivial reloads).
      x8 is host-normalized fp8 so q is already the cosine logits.
    - e = exp(q) on ACT straight off PSUM -> bf16 SBUF.
    - apply: per 128-row tile, out_ps[r, 0:256] = sum_c e[c,r]*delta[c,:]
      with stationary e-tile (K=16 -> fast weight load) and moving
      [delta | 1 | 0] -- the ones column emits the softmax denominator as
      output column 256 for free (no transposes, no reduce).
    - rden = 1/out_ps[:,256] (DVE reciprocal, per-partition scalar).
    - ob[r, d] = out_ps[r, d]*rden[r] + xrow[r, d] in ONE
      scalar_tensor_tensor, alternated between DVE and GpSimd.
  The few-shot segment reduction runs replicated on every core (an
  AllReduce costs a ~41us all-core barrier on this fabric): fp8 DoubleRow
  one-hot matmuls with rhs [x1 | 1 | (x2-x1)], accumulated over 40
  256-row pairs into two PSUM tiles; class stats (centers, cosine-
  normalized cn8T, delta) are computed once and stay resident.
"""

import os
from contextlib import ExitStack

import numpy as np
import ml_dtypes

import concourse.bass as bass
import concourse.mybir as mybir
import concourse.tile as tile
from concourse.bacc import Bacc

DT = mybir.dt
ALU = mybir.AluOpType
ACTF = mybir.ActivationFunctionType
BF = DT.bfloat16
FP8 = DT.float8e4

CORES = 8
N, D, NUM = 200000, 256, 16
S = 10000
S_PAIRS = 40                # few-shot 256-row tile-pairs (10240 padded rows)
FS_W = 514                  # [x1(256) | ones(1) | dx(256) | pad(1)]
FS_CH = 10                  # few-shot tile-pairs per DMA chunk
CHUNK = 256                 # main-loop rows per pipeline iteration


def build_nc(rp):
    nt = rp // 128            # 128-row tiles
    nch = rp // CHUNK         # 256-row chunks
    stt_dve = int(os.environ.get("KDBG_STT_DVE", 2))  # 1 of every k STTs on DVE

    nc = Bacc(target_bir_lowering=False, num_devices=CORES)

    x8d = nc.declare_dram_parameter("x8d", [128, 2, rp], FP8, isOutput=False)
    xrow = nc.declare_dram_parameter("xrow", [128, nt, 256], BF, isOutput=False)
    x12 = nc.declare_dram_parameter("x12", [128, S_PAIRS, 2, FS_W], FP8,
                                    isOutput=False)
    yf = nc.declare_dram_parameter("yf", [128, S_PAIRS, 2], DT.float32,
                                   isOutput=False)
    out = nc.declare_dram_parameter("out", [128, nt, 256], BF, isOutput=True)

    with tile.TileContext(nc) as tc, ExitStack() as ctx:
        cpool = ctx.enter_context(tc.tile_pool(name="const", bufs=1))

        # ---- constants ----
        ident_f = cpool.tile([128, 128], DT.float32)
        from concourse.masks import make_identity
        make_identity(nc, ident_f[:])
        ident_bf = cpool.tile([128, 128], BF)
        nc.vector.tensor_copy(ident_bf[:], ident_f[:])
        iota_i = cpool.tile([128, NUM], DT.int32)
        nc.gpsimd.iota(iota_i[:], pattern=[[1, NUM]], base=0, channel_multiplier=0)
        iota_f = cpool.tile([128, NUM], DT.float32)
        nc.vector.tensor_copy(iota_f[:], iota_i[:])
        yf_sb = cpool.tile([128, S_PAIRS, 2], DT.float32)
        nc.sync.dma_start(out=yf_sb[:], in_=yf[:, :, :])

        # resident main-loop inputs (fp8 d-major for logits, bf16 row-major
        # for the residual); issue few-shot chunks first, then interleave
        x8_all = ctx.enter_context(tc.tile_pool(name="x8a", bufs=1)).tile(
            [128, 2, rp], FP8, name="x8_all")
        xr_all = ctx.enter_context(tc.tile_pool(name="xra", bufs=1)).tile(
            [128, nt, 256], BF, name="xr_all")

        cn8T = cpool.tile([128, 2, NUM], FP8)
        delta_aug = cpool.tile([NUM, 258], BF)

        # ---- phase 1: few-shot per-class segment sums (replicated) ----
        with tc.tile_pool(name="fsp", bufs=1, space="PSUM") as fsps, \
             tc.tile_pool(name="fs", bufs=3) as fsp:
            cs1_ps = fsps.tile([NUM, 257], DT.float32, name="cs1_ps")
            cs2_ps = fsps.tile([NUM, 256], DT.float32, name="cs2_ps")
            DR = mybir.MatmulPerfMode.DoubleRow
            n_fs_ch = S_PAIRS // FS_CH
            for ch in range(n_fs_ch):
                a0 = ch * FS_CH
                x_c = fsp.tile([128, FS_CH, 2, FS_W], FP8, name="x_c")
                nc.sync.dma_start(out=x_c[:], in_=x12[:, a0:a0 + FS_CH, :, :])
                oh_c = fsp.tile([128, FS_CH, 2, NUM], FP8, name="oh_c")
                nc.vector.tensor_tensor(
                    out=oh_c[:],
                    in0=yf_sb[:, a0:a0 + FS_CH, :, None]
                        .to_broadcast([128, FS_CH, 2, NUM]),
                    in1=iota_f[:, None, None, :]
                        .to_broadcast([128, FS_CH, 2, NUM]),
                    op=ALU.is_equal)
                for a in range(FS_CH):
                    st = (a0 + a == 0)
                    sp = (a0 + a == S_PAIRS - 1)
                    nc.tensor.matmul(cs1_ps[:], lhsT=oh_c[:, a, :, :],
                                     rhs=x_c[:, a, :, 0:257], start=st, stop=sp,
                                     perf_mode=DR)
                    nc.tensor.matmul(cs2_ps[:], lhsT=oh_c[:, a, :, :],
                                     rhs=x_c[:, a, :, 257:513], start=st,
                                     stop=sp, perf_mode=DR)

            # ---- phase 2: class stats ----
            rc = cpool.tile([NUM, 1], DT.float32)
            nc.vector.reciprocal(rc[:], cs1_ps[:, 256:257])
            centers = cpool.tile([NUM, D], DT.float32)
            nc.vector.tensor_scalar_mul(centers[:], cs1_ps[:, 0:256], rc[:])
            nc.vector.tensor_scalar_mul(delta_aug[:, 0:256], cs2_ps[:, 0:256],
                                        rc[:])
            nc.vector.memset(delta_aug[:, 256:257], 1.0)
            nc.vector.memset(delta_aug[:, 257:258], 0.0)
            cscr = cpool.tile([NUM, D], DT.float32)
            nc.vector.tensor_tensor(
                out=cscr[:], in0=centers[:], in1=centers[:], op=ALU.mult)
            csum = cpool.tile([NUM, 1], DT.float32)
            nc.vector.tensor_reduce(
                out=csum[:], in_=cscr[:], axis=mybir.AxisListType.X, op=ALU.add)
            clog = cpool.tile([NUM, 1], DT.float32)
            nc.scalar.activation(out=clog[:], in_=csum[:], func=ACTF.Ln)
            cinv = cpool.tile([NUM, 1], DT.float32)
            nc.scalar.activation(out=cinv[:], in_=clog[:], func=ACTF.Exp,
                                 scale=-0.5)
            cn_bf = cpool.tile([NUM, D], BF)
            nc.vector.tensor_scalar_mul(cn_bf[:], centers[:], cinv[:])
            ctp = fsps.tile([128, 2, NUM], BF, name="ctp")
            for h in range(2):
                nc.tensor.transpose(ctp[:, h, :],
                                    in_=cn_bf[:, h * 128:(h + 1) * 128],
                                    identity=ident_bf[0:NUM, 0:NUM])
            nc.vector.tensor_copy(cn8T[:], ctp[:])

        # resident-table DMA: stream x8 (logits) and xrow (residual) in
        # paired 512-row groups so every chunk's operands land in step
        for g in range(rp // 512):
            nc.sync.dma_start(out=x8_all[:, :, g * 512:(g + 1) * 512],
                              in_=x8d[:, :, g * 512:(g + 1) * 512])
            nc.sync.dma_start(out=xr_all[:, 4 * g:4 * g + 4, :],
                              in_=xrow[:, 4 * g:4 * g + 4, :])

        # ---- phase 3: streaming main loop, 512-row groups ----
        qps = ctx.enter_context(tc.tile_pool(name="qps", bufs=2, space="PSUM"))
        cps = ctx.enter_context(tc.tile_pool(name="cps", bufs=3, space="PSUM"))
        smp = ctx.enter_context(tc.tile_pool(name="sm", bufs=4))
        rpp = ctx.enter_context(tc.tile_pool(name="rp", bufs=6))
        scp = ctx.enter_context(tc.tile_pool(name="sc", bufs=6))
        obp = ctx.enter_context(tc.tile_pool(name="ob", bufs=3))

        ngr = rp // 512
        stash = {}

        def stage_p0(g):          # PE: logits for 512 rows, stationary cn8T
            q = qps.tile([NUM, 512], DT.float32, name="q")
            nc.tensor.matmul(
                q[:], lhsT=cn8T[:], rhs=x8_all[:, :, g * 512:(g + 1) * 512],
                start=True, stop=True, perf_mode=mybir.MatmulPerfMode.DoubleRow)
            stash[g] = {"q": q}

        def stage_p1(g):          # ACT: exp off PSUM
            st = stash[g]
            e8 = smp.tile([NUM, 512], BF, name="e8")
            nc.scalar.activation(out=e8[:], in_=st.pop("q")[:], func=ACTF.Exp)
            st["e8"] = e8

        def stage_p2(g):          # PE: apply+den; DVE: rden; DVE/ACT+Pool: out
            e8 = stash.pop(g)["e8"]
            ob = obp.tile([128, 4, 256], BF, name="ob")
            for h in range(2):
                co = cps.tile([128, 2, 512], DT.float32, name="co")
                for j in range(2):
                    nc.tensor.matmul(co[:, j, 0:258],
                                     lhsT=e8[:, (2 * h + j) * 128:
                                             (2 * h + j + 1) * 128],
                                     rhs=delta_aug[:], start=True, stop=True)
                rden = rpp.tile([128, 2], DT.float32, name="rden")
                nc.vector.reciprocal(rden[:], co[:, :, 256])
                for j in range(2):
                    jj = 2 * h + j
                    t = 4 * g + jj
                    if t % stt_dve == 0:
                        # fused scale+add straight off PSUM on DVE
                        nc.vector.scalar_tensor_tensor(
                            out=ob[:, jj, :], in0=co[:, j, 0:256],
                            scalar=rden[:, j:j + 1], in1=xr_all[:, t, :],
                            op0=ALU.mult, op1=ALU.add)
                    else:
                        # ACT applies rden while draining PSUM; Pool adds xr
                        sc = scp.tile([128, 256], BF, name="sc")
                        nc.scalar.activation(out=sc[:], in_=co[:, j, 0:256],
                                             func=ACTF.Copy,
                                             scale=rden[:, j:j + 1])
                        nc.gpsimd.tensor_tensor(
                            out=ob[:, jj, :], in0=sc[:],
                            in1=xr_all[:, t, :], op=ALU.add)
            nc.sync.dma_start(out=out[:, g * 4:g * 4 + 4, :], in_=ob[:])

        stages = [(0, stage_p0), (1, stage_p1), (2, stage_p2)]
        depth = stages[-1][0] + 1
        for it in range(ngr + depth - 1):
            for off, fn in stages:
                kk = it - off
                if 0 <= kk < ngr:
                    fn(kk)
    nc.finalize()
    return nc


def _shard_inputs(Q1_x, Q2_x, Q1_y, selected_idxes, remaining_idxes):
    """Host-side glue: few-shot layout, dedup of the remaining-row support,
    value-range sharding of the unique rows across cores."""
    bf16 = ml_dtypes.bfloat16
    fp8 = ml_dtypes.float8_e4m3
    Q1_x = np.asarray(Q1_x, dtype=np.float32)
    Q2_x = np.asarray(Q2_x, dtype=np.float32)
    y = np.asarray(Q1_y).astype(np.int32)
    sel = np.asarray(selected_idxes).astype(np.int64)
    rem = np.asarray(remaining_idxes).astype(np.int64)

    uniq, inv = np.unique(rem, return_inverse=True)
    bounds = np.searchsorted(uniq, np.arange(CORES + 1) * (N // CORES))
    ncounts = np.diff(bounds)
    gran = 2 * OBROWS
    rp = int(max(1, -(-int(ncounts.max()) // gran))) * gran

    # few-shot block: [x1(256) | ones | x2-x1(256) | pad]
    s_pad = S_PAIRS * 256
    v = np.zeros((s_pad, FS_W), dtype=np.float32)
    v[:S, 0:256] = Q1_x[sel]
    v[:S, 256] = 1.0
    v[:S, 257:513] = Q2_x[sel] - Q1_x[sel]
    x12 = np.ascontiguousarray(
        v.reshape(S_PAIRS, 2, 128, FS_W).transpose(2, 0, 1, 3).astype(fp8))
    yv = np.full((s_pad,), -1.0, dtype=np.float32)
    yv[:S] = y[sel].astype(np.float32)
    yfa = np.ascontiguousarray(yv.reshape(S_PAIRS, 2, 128).transpose(2, 0, 1))

    in_maps = []
    for c in range(CORES):
        rows_c = uniq[bounds[c]:bounds[c + 1]]
        xs = np.ones((rp, D), dtype=np.float32)
        xs[:len(rows_c)] = Q1_x[rows_c]
        rn = 1.0 / np.sqrt(np.einsum("rd,rd->r", xs, xs))
        # x8d[p, h, r] = xn[r, h*128+p]
        x8 = np.ascontiguousarray(
            (xs * rn[:, None]).T.reshape(2, 128, rp).transpose(1, 0, 2)
            .astype(fp8))
        # xrow[p, t, d] = xs[t*128+p, d]
        xr = np.ascontiguousarray(
            xs.reshape(rp // 128, 128, D).transpose(1, 0, 2).astype(bf16))
        in_maps.append({"x8d": x8, "xrow": xr, "x12": x12, "yf": yfa})
    return in_maps, rp, bounds, inv, len(uniq)


OBROWS = 256  # rows per ob granule (must match OB_CH * CHUNK inside build_nc)


def kernel(Q1_x, Q2_x, Q1_y, selected_idxes, remaining_idxes, num, _bench=None):
    from concourse.bass_utils import run_bass_kernel_spmd

    in_maps, rp, bounds, inv, nuniq = _shard_inputs(
        Q1_x, Q2_x, Q1_y, selected_idxes, remaining_idxes)
    nc = build_nc(rp)
    kwargs = dict(_bench or {})
    res = run_bass_kernel_spmd(nc, in_maps, core_ids=list(range(CORES)), **kwargs)
    full = np.empty((nuniq, D), dtype=np.float32)
    for c in range(CORES):
        blk = np.asarray(res.results[c]["out"])  # [128, nt, 256] row-major
        n_c = bounds[c + 1] - bounds[c]
        full[bounds[c]:bounds[c + 1]] = (
            blk.transpose(1, 0, 2).reshape(rp, D)[:n_c].astype(np.float32))
    out = full[inv]
    if _bench is not None:
        kernel.last_results = res
    return out


# revision 14
# speedup vs baseline: 1.0235x; 1.0235x over previous
"""Trainium2 Bass kernel for the AdaptPrompt segment-reduce problem.

Computation (see reference):
    counts/centers/delta = per-class segment means over 10000 few-shot rows
    xr = Q1_x[remaining_idxes]                       # [190000, 256] gather
    sim = softmax(normalize(xr) @ normalize(centers).T)
    out = xr + sim @ delta

Strategy (row-major streaming, constant PE stationaries):
  Host dedups the remaining-row support (~61% of table rows are referenced)
  and value-range shards the unique rows across 8 cores; out[i] =
  dev_out[rem[i]] is applied on the host after the gather-free streaming
  kernel.

  Device pipeline per 256-row chunk:
    - logits  q[c, r] = cn8^T @ x8 on PE, fp8 DoubleRow, with the class
      matrix cn8T as the stationary operand (constant -> trivial reloads).
      x8 is host-normalized fp8 so q is already the cosine logits.
    - e = exp(q) on ACT straight off PSUM -> bf16 SBUF.
    - apply: per 128-row tile, out_ps[r, 0:256] = sum_c e[c,r]*delta[c,:]
      with stationary e-tile (K=16 -> fast weight load) and moving
      [delta | 1 | 0] -- the ones column emits the softmax denominator as
      output column 256 for free (no transposes, no reduce).
    - rden = 1/out_ps[:,256] (DVE reciprocal, per-partition scalar).
    - ob[r, d] = out_ps[r, d]*rden[r] + xrow[r, d] in ONE
      scalar_tensor_tensor, alternated between DVE and GpSimd.
  The few-shot segment reduction runs replicated on every core (an
  AllReduce costs a ~41us all-core barrier on this fabric): fp8 DoubleRow
  one-hot matmuls with rhs [x1 | 1 | (x2-x1)], accumulated over 40
  256-row pairs into two PSUM tiles; class stats (centers, cosine-
  normalized cn8T, delta) are computed once and stay resident.
"""

import os
from contextlib import ExitStack

import numpy as np
import ml_dtypes

import concourse.bass as bass
import concourse.mybir as mybir
import concourse.tile as tile
from concourse.bacc import Bacc

DT = mybir.dt
ALU = mybir.AluOpType
ACTF = mybir.ActivationFunctionType
BF = DT.bfloat16
FP8 = DT.float8e4

CORES = 8
N, D, NUM = 200000, 256, 16
S = 10000
S_PAIRS = 40                # few-shot 256-row tile-pairs (10240 padded rows)
FS_W = 514                  # [x1(256) | ones(1) | dx(256) | pad(1)]
FS_CH = 10                  # few-shot tile-pairs per DMA chunk
CHUNK = 256                 # main-loop rows per pipeline iteration


def build_nc(rp):
    nt = rp // 128            # 128-row tiles
    nch = rp // CHUNK         # 256-row chunks
    stt_dve = int(os.environ.get("KDBG_STT_DVE", 2))  # 1 of every k STTs on DVE

    nc = Bacc(target_bir_lowering=False, num_devices=CORES)

    x8d = nc.declare_dram_parameter("x8d", [128, 2, rp], FP8, isOutput=False)
    xrow = nc.declare_dram_parameter("xrow", [128, nt, 256], BF, isOutput=False)
    x12 = nc.declare_dram_parameter("x12", [128, S_PAIRS, 2, FS_W], FP8,
                                    isOutput=False)
    yf = nc.declare_dram_parameter("yf", [128, S_PAIRS, 2], DT.float32,
                                   isOutput=False)
    out = nc.declare_dram_parameter("out", [128, nt, 256], BF, isOutput=True)

    with tile.TileContext(nc) as tc, ExitStack() as ctx:
        cpool = ctx.enter_context(tc.tile_pool(name="const", bufs=1))

        # ---- constants ----
        ident_f = cpool.tile([128, 128], DT.float32)
        from concourse.masks import make_identity
        make_identity(nc, ident_f[:])
        ident_bf = cpool.tile([128, 128], BF)
        nc.vector.tensor_copy(ident_bf[:], ident_f[:])
        iota_i = cpool.tile([128, NUM], DT.int32)
        nc.gpsimd.iota(iota_i[:], pattern=[[1, NUM]], base=0, channel_multiplier=0)
        iota_f = cpool.tile([128, NUM], DT.float32)
        nc.vector.tensor_copy(iota_f[:], iota_i[:])
        yf_sb = cpool.tile([128, S_PAIRS, 2], DT.float32)
        nc.sync.dma_start(out=yf_sb[:], in_=yf[:, :, :])

        # resident main-loop inputs (fp8 d-major for logits, bf16 row-major
        # for the residual); issue few-shot chunks first, then interleave
        x8_all = ctx.enter_context(tc.tile_pool(name="x8a", bufs=1)).tile(
            [128, 2, rp], FP8, name="x8_all")
        xr_all = ctx.enter_context(tc.tile_pool(name="xra", bufs=1)).tile(
            [128, nt, 256], BF, name="xr_all")

        cn8T = cpool.tile([128, 2, NUM], FP8)
        # augmented moving operand for the apply: [delta | 1 | 0] -- the
        # ones column emits the softmax denominator as output column 256
        delta_aug = cpool.tile([NUM, 258], BF)

        # ---- phase 1: few-shot per-class segment sums (replicated) ----
        with tc.tile_pool(name="fsp", bufs=1, space="PSUM") as fsps, \
             tc.tile_pool(name="fs", bufs=3) as fsp:
            cs1_ps = fsps.tile([NUM, 257], DT.float32, name="cs1_ps")
            cs2_ps = fsps.tile([NUM, 256], DT.float32, name="cs2_ps")
            DR = mybir.MatmulPerfMode.DoubleRow
            n_fs_ch = S_PAIRS // FS_CH
            for ch in range(n_fs_ch):
                a0 = ch * FS_CH
                x_c = fsp.tile([128, FS_CH, 2, FS_W], FP8, name="x_c")
                nc.sync.dma_start(out=x_c[:], in_=x12[:, a0:a0 + FS_CH, :, :])
                oh_c = fsp.tile([128, FS_CH, 2, NUM], FP8, name="oh_c")
                nc.vector.tensor_tensor(
                    out=oh_c[:],
                    in0=yf_sb[:, a0:a0 + FS_CH, :, None]
                        .to_broadcast([128, FS_CH, 2, NUM]),
                    in1=iota_f[:, None, None, :]
                        .to_broadcast([128, FS_CH, 2, NUM]),
                    op=ALU.is_equal)
                for a in range(FS_CH):
                    st = (a0 + a == 0)
                    sp = (a0 + a == S_PAIRS - 1)
                    nc.tensor.matmul(cs1_ps[:], lhsT=oh_c[:, a, :, :],
                                     rhs=x_c[:, a, :, 0:257], start=st, stop=sp,
                                     perf_mode=DR)
                    nc.tensor.matmul(cs2_ps[:], lhsT=oh_c[:, a, :, :],
                                     rhs=x_c[:, a, :, 257:513], start=st,
                                     stop=sp, perf_mode=DR)

            # ---- phase 2: class stats ----
            rc = cpool.tile([NUM, 1], DT.float32)
            nc.vector.reciprocal(rc[:], cs1_ps[:, 256:257])
            centers = cpool.tile([NUM, D], DT.float32)
            nc.vector.tensor_scalar_mul(centers[:], cs1_ps[:, 0:256], rc[:])
            nc.vector.tensor_scalar_mul(delta_aug[:, 0:256], cs2_ps[:, 0:256],
                                        rc[:])
            nc.vector.memset(delta_aug[:, 256:257], 1.0)
            nc.vector.memset(delta_aug[:, 257:258], 0.0)
            cscr = cpool.tile([NUM, D], DT.float32)
            nc.vector.tensor_tensor(
                out=cscr[:], in0=centers[:], in1=centers[:], op=ALU.mult)
            csum = cpool.tile([NUM, 1], DT.float32)
            nc.vector.tensor_reduce(
                out=csum[:], in_=cscr[:], axis=mybir.AxisListType.X, op=ALU.add)
            clog = cpool.tile([NUM, 1], DT.float32)
            nc.scalar.activation(out=clog[:], in_=csum[:], func=ACTF.Ln)
            cinv = cpool.tile([NUM, 1], DT.float32)
            nc.scalar.activation(out=cinv[:], in_=clog[:], func=ACTF.Exp,
                                 scale=-0.5)
            cn_bf = cpool.tile([NUM, D], BF)
            nc.vector.tensor_scalar_mul(cn_bf[:], centers[:], cinv[:])
            ctp = fsps.tile([128, 2, NUM], BF, name="ctp")
            for h in range(2):
                nc.tensor.transpose(ctp[:, h, :],
                                    in_=cn_bf[:, h * 128:(h + 1) * 128],
                                    identity=ident_bf[0:NUM, 0:NUM])
            nc.vector.tensor_copy(cn8T[:], ctp[:])

        # resident-table DMA: stream x8 (logits) and xrow (residual) in
        # paired 2048-row super-chunks so operands land in step with use
        for g in range((rp + 2047) // 2048):
            c0 = g * 2048
            w = min(rp - c0, 2048)
            nc.sync.dma_start(out=x8_all[:, :, c0:c0 + w],
                              in_=x8d[:, :, c0:c0 + w])
            for h in range(2):
                t0 = 16 * g + 8 * h
                wt = min(nt - t0, 8)
                if wt > 0:
                    nc.sync.dma_start(out=xr_all[:, t0:t0 + wt, :],
                                      in_=xrow[:, t0:t0 + wt, :])

        # ---- phase 3: streaming main loop, 512-row groups ----
        # Deep software pipeline (emission deepest-stage-first): every
        # cross-engine dependency is produced >=1 iteration before its
        # consumer so no engine queue head-of-line blocks.
        qps = ctx.enter_context(tc.tile_pool(name="qps", bufs=2, space="PSUM"))
        cps = ctx.enter_context(tc.tile_pool(name="cps", bufs=3, space="PSUM"))
        smp = ctx.enter_context(tc.tile_pool(name="sm", bufs=3))
        rpp = ctx.enter_context(tc.tile_pool(name="rp", bufs=4))
        scp = ctx.enter_context(tc.tile_pool(name="sc", bufs=8))
        obp = ctx.enter_context(tc.tile_pool(name="ob", bufs=3))

        ngr = rp // 512
        stash = {}
        ob_tiles = {}
        # per co-pair drain mode, cycled: D = fused DVE STT off PSUM,
        # U = ACT bulk-copy to SBUF + DVE bf16 STTs, C = ACT scale-copy
        # per tile + Pool adds
        drain_pat = os.environ.get("KDBG_DRAIN", "DDCDUDCDDCDUDCDC")

        def stage_p0(g):          # PE: logits, stationary cn8T
            q = qps.tile([NUM, 512], DT.float32, name="q")
            nc.tensor.matmul(
                q[:], lhsT=cn8T[:], rhs=x8_all[:, :, g * 512:(g + 1) * 512],
                start=True, stop=True, perf_mode=mybir.MatmulPerfMode.DoubleRow)
            stash[g] = q

        def stage_p1(g):          # ACT: exp off PSUM
            e8 = smp.tile([NUM, 512], BF, name="e8")
            nc.scalar.activation(out=e8[:], in_=stash.pop(g)[:], func=ACTF.Exp)
            stash[(g, "e")] = e8

        def stage_p2(g):          # PE: apply + den via ones-column
            e8 = stash.pop((g, "e"))
            cos = []
            for h in range(2):
                co = cps.tile([128, 2, 512], DT.float32, name="co")
                for j in range(2):
                    nc.tensor.matmul(
                        co[:, j, 0:258],
                        lhsT=e8[:, (2 * h + j) * 128:(2 * h + j + 1) * 128],
                        rhs=delta_aug[:], start=True, stop=True)
                cos.append(co)
            stash[g] = cos

        def stage_p3(g):          # DVE: rden; drain co per pattern
            cos = stash.pop(g)
            ob = obp.tile([128, 4, 256], BF, name="ob")
            ob_tiles[g] = ob
            adds = []
            for h in range(2):
                co = cos[h]
                mode = drain_pat[(2 * g + h) % len(drain_pat)]
                rden = rpp.tile([128, 2], DT.float32, name="rden")
                nc.vector.reciprocal(rden[:], co[:, :, 256])
                if mode == "D":
                    for j in range(2):
                        nc.vector.scalar_tensor_tensor(
                            out=ob[:, 2 * h + j, :], in0=co[:, j, 0:256],
                            scalar=rden[:, j:j + 1],
                            in1=xr_all[:, 4 * g + 2 * h + j, :],
                            op0=ALU.mult, op1=ALU.add)
                elif mode == "U":
                    sc = scp.tile([128, 2, 256], BF, name="sc")
                    nc.scalar.copy(sc[:], co[:, :, 0:256])
                    for j in range(2):
                        nc.vector.scalar_tensor_tensor(
                            out=ob[:, 2 * h + j, :], in0=sc[:, j, :],
                            scalar=rden[:, j:j + 1],
                            in1=xr_all[:, 4 * g + 2 * h + j, :],
                            op0=ALU.mult, op1=ALU.add)
                else:  # C
                    for j in range(2):
                        sc = scp.tile([128, 2, 256], BF, name="scc")
                        nc.scalar.activation(out=sc[:, 0, :],
                                             in_=co[:, j, 0:256],
                                             func=ACTF.Copy,
                                             scale=rden[:, j:j + 1])
                        adds.append((2 * h + j, sc))
            stash[(g, "adds")] = adds

        def stage_p4(g):          # Pool: deferred adds; SP: output DMA
            ob = ob_tiles.pop(g)
            for jj, sc in stash.pop((g, "adds")):
                nc.gpsimd.tensor_tensor(
                    out=ob[:, jj, :], in0=sc[:, 0, :],
                    in1=xr_all[:, 4 * g + jj, :], op=ALU.add)
            nc.sync.dma_start(out=out[:, g * 4:g * 4 + 4, :], in_=ob[:])

        stages = [(4, stage_p4), (3, stage_p3), (2, stage_p2), (1, stage_p1),
                  (0, stage_p0)]
        depth = stages[0][0] + 1
        for it in range(ngr + depth - 1):
            for off, fn in stages:   # deepest stage first
                kk = it - off
                if 0 <= kk < ngr:
                    fn(kk)
    nc.finalize()
    return nc


def _shard_inputs(Q1_x, Q2_x, Q1_y, selected_idxes, remaining_idxes):
    """Host-side glue: few-shot layout, dedup of the remaining-row support,
    value-range sharding of the unique rows across cores."""
    bf16 = ml_dtypes.bfloat16
    fp8 = ml_dtypes.float8_e4m3
    Q1_x = np.asarray(Q1_x, dtype=np.float32)
    Q2_x = np.asarray(Q2_x, dtype=np.float32)
    y = np.asarray(Q1_y).astype(np.int32)
    sel = np.asarray(selected_idxes).astype(np.int64)
    rem = np.asarray(remaining_idxes).astype(np.int64)

    uniq, inv = np.unique(rem, return_inverse=True)
    bounds = np.searchsorted(uniq, np.arange(CORES + 1) * (N // CORES))
    ncounts = np.diff(bounds)
    gran = 2 * OBROWS
    rp = int(max(1, -(-int(ncounts.max()) // gran))) * gran

    # few-shot block: [x1(256) | ones | x2-x1(256) | pad]
    s_pad = S_PAIRS * 256
    v = np.zeros((s_pad, FS_W), dtype=np.float32)
    v[:S, 0:256] = Q1_x[sel]
    v[:S, 256] = 1.0
    v[:S, 257:513] = Q2_x[sel] - Q1_x[sel]
    x12 = np.ascontiguousarray(
        v.reshape(S_PAIRS, 2, 128, FS_W).transpose(2, 0, 1, 3).astype(fp8))
    yv = np.full((s_pad,), -1.0, dtype=np.float32)
    yv[:S] = y[sel].astype(np.float32)
    yfa = np.ascontiguousarray(yv.reshape(S_PAIRS, 2, 128).transpose(2, 0, 1))

    in_maps = []
    for c in range(CORES):
        rows_c = uniq[bounds[c]:bounds[c + 1]]
        xs = np.ones((rp, D), dtype=np.float32)
        xs[:len(rows_c)] = Q1_x[rows_c]
        rn = 1.0 / np.sqrt(np.einsum("rd,rd->r", xs, xs))
        # x8d[p, h, r] = xn[r, h*128+p]
        x8 = np.ascontiguousarray(
            (xs * rn[:, None]).T.reshape(2, 128, rp).transpose(1, 0, 2)
            .astype(fp8))
        # xrow[p, t, d] = xs[t*128+p, d]
        xr = np.ascontiguousarray(
            xs.reshape(rp // 128, 128, D).transpose(1, 0, 2).astype(bf16))
        in_maps.append({"x8d": x8, "xrow": xr, "x12": x12, "yf": yfa})
    return in_maps, rp, bounds, inv, len(uniq)


OBROWS = 256  # rows per ob granule (must match OB_CH * CHUNK inside build_nc)


def kernel(Q1_x, Q2_x, Q1_y, selected_idxes, remaining_idxes, num, _bench=None):
    from concourse.bass_utils import run_bass_kernel_spmd

    in_maps, rp, bounds, inv, nuniq = _shard_inputs(
        Q1_x, Q2_x, Q1_y, selected_idxes, remaining_idxes)
    nc = build_nc(rp)
    kwargs = dict(_bench or {})
    res = run_bass_kernel_spmd(nc, in_maps, core_ids=list(range(CORES)), **kwargs)
    full = np.empty((nuniq, D), dtype=np.float32)
    for c in range(CORES):
        blk = np.asarray(res.results[c]["out"])  # [128, nt, 256] row-major
        n_c = bounds[c + 1] - bounds[c]
        full[bounds[c]:bounds[c + 1]] = (
            blk.transpose(1, 0, 2).reshape(rp, D)[:n_c].astype(np.float32))
    out = full[inv]
    if _bench is not None:
        kernel.last_results = res
    return out
